# revision 30
# baseline (speedup 1.0000x reference)
"""Trainium2 Bass kernel for BehaviorLemming (two fused stencil steps).

Sharding: data-parallel over batch. B=16 across 8 cores -> 2 batches/core.
Layout: H rows in partitions, (channel, W) in the free dim; input is
streamed per 4-channel group (1MB DMAs) so sets pipeline smoothly.
Per row-tile: DVE computes masks and the products P=a*w (Q=b*w on
GPSIMD); PE applies the +-1 row shifts as bit-exact fp32 matmuls with
shifted identity matrices, accumulating S_up@Q + S_dn@P in PSUM; the
"no move" case is patched with copy_predicated (uint8 m0 mask) after
ScalarE evacuates PSUM. Both steps run on-chip; the intermediate world
never touches HBM.

H tiling: 4 main sets of 124 output rows per batch (128 input rows incl.
2-row circular halo each side), plus ONE merged set handling the last 16
rows of BOTH batches (b0 at partitions 0..19, b1 at 32..51, block-
diagonal shift matrices).
"""

import numpy as np

_PQPOOL = [None]

import concourse.bacc as bacc
import concourse.mybir as mybir
import concourse.tile as tile
from concourse.bass_utils import run_bass_kernel_spmd

B, C, H, W = 16, 20, 512, 512
N_CORES = 8
B_PER_CORE = B // N_CORES
ELEM_ID = 3.0
F32 = mybir.dt.float32
U8 = mybir.dt.uint8
NCH = 4                 # channels per PSUM group (4 banks; bufs=2 -> 8)
NGRP = C // NCH
GP_Q = True             # Q products go to GPSIMD
MAIN_OUT = 124          # output rows per main set
MERGED_B1_OFF = 32      # partition offset of batch 1 rows in the merged set
MERGED_NP = 52
DMA_SHIFT_GROUPS = (1, 3)   # step-2 groups whose shifts ride DMA instead of PE


def _load_rows(nc, dst_tile, src_ap, row_start, n_rows, p0=0):
    """Load n_rows (mod H, split at wrap) of src [NCH,H,W] into dst
    partitions [p0, p0+n_rows), free dim = (c, w)."""
    s = row_start % H
    remaining = n_rows
    while remaining > 0:
        n = min(remaining, H - s)
        src = src_ap[:, s : s + n, :].rearrange("c h w -> h c w")
        nc.sync.dma_start(out=dst_tile[p0 : p0 + n, :].rearrange(
            "h (c w) -> h c w", c=NCH), in_=src)
        p0 += n
        s = (s + n) % H
        remaining -= n


def _build_masks(nc, pool, pmain, su, sd, world_t, np_, shift_w):
    """Masks for one step. world_t's free dim starts with ch0 (elem ids)
    then ch1 (density). Returns (a_f32, b_f32, m0_u8) SBUF tiles."""
    al = mybir.AluOpType
    e = world_t[0:np_, 0:W]
    d = world_t[0:np_, W : 2 * W]

    # dR = roll(d, shift_w) along the free (W) axis
    dR = pool.tile([np_, W], F32, tag="dR")
    if shift_w == 1:
        nc.scalar.copy(dR[:, 1:W], d[:, 0 : W - 1])
        nc.scalar.copy(dR[:, 0:1], d[:, W - 1 : W])
    else:
        nc.scalar.copy(dR[:, 0 : W - 1], d[:, 1:W])
        nc.scalar.copy(dR[:, W - 1 : W], d[:, 0:1])

    # mask shift matmuls share one pmain slot: dA | dAR | b
    mp = pmain.tile([np_, NCH * W], F32, tag="ps")
    dA = mp[:, 0:W]
    dAR = mp[:, W : 2 * W]
    bp = mp[:, 2 * W : 3 * W]
    nc.tensor.matmul(out=dA, lhsT=su, rhs=d, start=True, stop=True)
    nc.tensor.matmul(out=dAR, lhsT=su, rhs=dR[:], start=True, stop=True)

    c1 = pool.tile([np_, W], F32, tag="c1")
    c2 = pool.tile([np_, W], F32, tag="c2")
    c3 = pool.tile([np_, W], F32, tag="c3")
    nc.vector.tensor_tensor(out=c1[:], in0=dR[:], in1=d, op=al.is_ge)
    nc.vector.tensor_tensor(out=c2[:], in0=dA, in1=d, op=al.is_lt)
    nc.vector.tensor_tensor(out=c3[:], in0=dAR, in1=d, op=al.is_lt)
    e3c3 = pool.tile([np_, W], F32, tag="e3")
    nc.vector.scalar_tensor_tensor(out=e3c3[:], in0=e, scalar=ELEM_ID,
                                   in1=c3[:], op0=al.is_equal,
                                   op1=al.logical_and)
    c12 = pool.tile([np_, W], F32, tag="c12")
    nc.vector.tensor_tensor(out=c12[:], in0=c1[:], in1=c2[:],
                            op=al.logical_and)
    a = pool.tile([np_, W], F32, tag="a")
    nc.vector.tensor_tensor(out=a[:], in0=c12[:], in1=e3c3[:],
                            op=al.logical_and)

    # b[p] = a[p+1]; evacuate to SBUF so the psum slot frees quickly
    nc.tensor.matmul(out=bp, lhsT=sd, rhs=a[:], start=True, stop=True)
    b = pool.tile([np_, W], F32, tag="b")
    nc.scalar.copy(b[:], bp)

    # m0 = (a | b) == 0, as uint8 for copy_predicated
    r = pool.tile([np_, W], F32, tag="r")
    nc.vector.tensor_tensor(out=r[:], in0=a[:], in1=b[:], op=al.logical_or)
    m0 = pool.tile([np_, W], U8, tag="m0")
    nc.vector.tensor_scalar(out=m0[:], in0=r[:], scalar1=0.0, scalar2=None,
                            op0=al.is_equal)
    return a, b, m0


def _step_combine(nc, pool, pmain, su, sd, src_g, a, b, m0, np_, dst_g,
                  shift_via_dma=False, q_on_dve=False):
    """One stencil step for one NCH-channel group:
    dst = m0 ? src : (S_up@(b*src) + S_dn@(a*src)).

    shift_via_dma: apply the row shifts with SBUF->SBUF accumulating DMAs
    instead of PE matmuls (dst rows 0 / np_-1 end up garbage; only legal
    when those rows are never consumed, i.e. step-2 output tiles)."""
    al = mybir.AluOpType
    fd = NCH * W
    src_v = src_g.rearrange("p (c w) -> p c w", c=NCH)
    a_b = a[:].unsqueeze(1).broadcast_to([np_, NCH, W])
    b_b = b[:].unsqueeze(1).broadcast_to([np_, NCH, W])
    m0_b = m0[:].unsqueeze(1).broadcast_to([np_, NCH, W])

    P = _PQPOOL[0].tile([np_, fd], F32, tag="P")
    Q = _PQPOOL[0].tile([np_, fd], F32, tag="Q")
    nc.vector.tensor_tensor(out=P[:].rearrange("p (c w) -> p c w", c=NCH),
                            in0=src_v, in1=a_b, op=al.mult)
    qeng = nc.vector if (q_on_dve or not GP_Q) else nc.gpsimd
    qeng.tensor_tensor(out=Q[:].rearrange("p (c w) -> p c w", c=NCH),
                       in0=src_v, in1=b_b, op=al.mult)

    if shift_via_dma:
        # dst[p] = Q[p-1]; then dst[p] += P[p+1]
        nc.gpsimd.dma_start(out=dst_g.tensor[1:np_, 0:fd],
                            in_=Q[0 : np_ - 1, :])
        nc.gpsimd.dma_start(out=dst_g.tensor[0 : np_ - 1, 0:fd],
                            in_=P[1:np_, :], accum_op=al.add)
    else:
        ps = pmain.tile([np_, fd], F32, tag="ps")
        for c in range(NCH):
            nc.tensor.matmul(out=ps[:, c * W : (c + 1) * W], lhsT=su,
                             rhs=Q[:, c * W : (c + 1) * W],
                             start=True, stop=False)
        for c in range(NCH):
            nc.tensor.matmul(out=ps[:, c * W : (c + 1) * W], lhsT=sd,
                             rhs=P[:, c * W : (c + 1) * W],
                             start=False, stop=True)
        nc.scalar.copy(dst_g, ps[:])
    nc.vector.copy_predicated(dst_g.rearrange("p (c w) -> p c w", c=NCH),
                              m0_b, src_v)


def _new_set_state(nc, pools, sd):
    """Allocate w1, load group 0 and build step-1 masks for a set."""
    wpool, bigpool, opool, pool, pmain = pools
    sup, sdn, np_, load_group, _sg = sd
    g0 = wpool.tile([128, NCH * W], F32, tag="w0g")
    load_group(g0, 0)
    masks1 = _build_masks(nc, pool, pmain, sup, sdn, g0, np_, 1)
    w1 = bigpool.tile([128, C * W], F32, tag="w1")
    return {"sd": sd, "g0": g0, "masks1": masks1, "w1": w1, "masks2": None}


def _step1_group(nc, pools, st, g):
    """Emit step-1 for one channel group of a set."""
    wpool, bigpool, opool, pool, pmain = pools
    sup, sdn, np_, load_group, _sg = st["sd"]
    a1, b1, m01 = st["masks1"]
    if g == 0:
        t = st["g0"]
    else:
        t = wpool.tile([128, NCH * W], F32, tag="w0g")
        load_group(t, g)
    dst = st["w1"][0:np_, g * NCH * W : (g + 1) * NCH * W]
    _step_combine(nc, pool, pmain, sup, sdn, t[0:np_, :],
                  a1, b1, m01, np_, dst, q_on_dve=(g == 0))
    if g == 0:
        # step-2 masks only need w1 ch0/ch1: emit now so the mask DVE
        # chain overlaps other groups' PE work
        st["masks2"] = _build_masks(nc, pool, pmain, sup, sdn,
                                    st["w1"], np_, -1)


def _step2_group(nc, pools, st, g):
    """Emit step-2 + store for one channel group of a set."""
    wpool, bigpool, opool, pool, pmain = pools
    sup, sdn, np_, _lg, store_group = st["sd"]
    a2, b2, m02 = st["masks2"]
    src = st["w1"][0:np_, g * NCH * W : (g + 1) * NCH * W]
    og = opool.tile([128, NCH * W], F32, tag="og")
    _step_combine(nc, pool, pmain, sup, sdn, src,
                  a2, b2, m02, np_, og[0:np_, :],
                  shift_via_dma=(g in DMA_SHIFT_GROUPS),
                  q_on_dve=(g == 0))
    store_group(og, g)


def build_kernel():
    nc = bacc.Bacc("TRN2", target_bir_lowering=False, debug=False,
                   num_devices=N_CORES)
    wd = nc.dram_tensor("world", [B_PER_CORE, C, H, W], F32,
                        kind="ExternalInput").ap()
    su_d = nc.dram_tensor("s_up", [128, 128], F32, kind="ExternalInput").ap()
    sd_d = nc.dram_tensor("s_dn", [128, 128], F32, kind="ExternalInput").ap()
    sum_d = nc.dram_tensor("s_up_m", [MERGED_NP, MERGED_NP], F32,
                           kind="ExternalInput").ap()
    sdm_d = nc.dram_tensor("s_dn_m", [MERGED_NP, MERGED_NP], F32,
                           kind="ExternalInput").ap()
    od = nc.dram_tensor("out", [B_PER_CORE, C, H, W], F32,
                        kind="ExternalOutput").ap()

    with tile.TileContext(nc) as tc:
        with (
            tc.tile_pool(name="const", bufs=1) as cpool,
            tc.tile_pool(name="wpool", bufs=3) as wpool,
            tc.tile_pool(name="big", bufs=2) as bigpool,
            tc.tile_pool(name="opool", bufs=2) as opool,
            tc.tile_pool(name="small", bufs=2) as pool,
            tc.tile_pool(name="pq", bufs=3) as pqpool,
            tc.tile_pool(name="pmain", bufs=2, space="PSUM") as pmain,
        ):
            st_up = cpool.tile([128, 128], F32)
            st_dn = cpool.tile([128, 128], F32)
            st_up_m = cpool.tile([MERGED_NP, MERGED_NP], F32)
            st_dn_m = cpool.tile([MERGED_NP, MERGED_NP], F32)
            nc.sync.dma_start(out=st_up[:], in_=su_d)
            nc.sync.dma_start(out=st_dn[:], in_=sd_d)
            nc.sync.dma_start(out=st_up_m[:], in_=sum_d)
            nc.sync.dma_start(out=st_dn_m[:], in_=sdm_d)

            _PQPOOL[0] = pqpool
            pools = (wpool, bigpool, opool, pool, pmain)

            def make_main_set(bi, si):
                r_out = si * MAIN_OUT

                def load_group(t, g):
                    src = wd[bi, g * NCH : (g + 1) * NCH]
                    _load_rows(nc, t, src, r_out - 2, 128)

                def store_group(og, g):
                    dst = od[bi, g * NCH : (g + 1) * NCH,
                             r_out : r_out + MAIN_OUT, :]
                    nc.sync.dma_start(
                        out=dst.rearrange("c h w -> h c w"),
                        in_=og[2 : 2 + MAIN_OUT, :].rearrange(
                            "h (c w) -> h c w", c=NCH))

                return (st_up[:], st_dn[:], 128, load_group, store_group)

            def make_merged_set():
                r_out = 4 * MAIN_OUT      # 496
                n_out = H - r_out         # 16

                def load_group(t, g):
                    # zero first (aligned range) so gap partitions between
                    # the batch blocks can't feed NaN garbage into the PE
                    nc.gpsimd.memset(t[0:64, :], 0.0)
                    for bi, p0 in ((0, 0), (1, MERGED_B1_OFF)):
                        src = wd[bi, g * NCH : (g + 1) * NCH]
                        _load_rows(nc, t, src, r_out - 2, n_out + 4, p0=p0)

                def store_group(og, g):
                    for bi, p0 in ((0, 2), (1, MERGED_B1_OFF + 2)):
                        dst = od[bi, g * NCH : (g + 1) * NCH,
                                 r_out : r_out + n_out, :]
                        nc.sync.dma_start(
                            out=dst.rearrange("c h w -> h c w"),
                            in_=og[p0 : p0 + n_out, :].rearrange(
                                "h (c w) -> h c w", c=NCH))

                return (st_up_m[:], st_dn_m[:], MERGED_NP, load_group,
                        store_group)

            sets = [make_main_set(bi, si)
                    for bi in range(B_PER_CORE) for si in range(4)]
            sets.append(make_merged_set())

            # software-pipelined emission: the NEXT set's g0 load + step-1
            # masks are emitted before the CURRENT set's step-2 groups
            st = _new_set_state(nc, pools, sets[0])
            _step1_group(nc, pools, st, 0)
            for i in range(len(sets)):
                for g in range(1, NGRP):
                    _step1_group(nc, pools, st, g)
                st_next = (_new_set_state(nc, pools, sets[i + 1])
                           if i + 1 < len(sets) else None)
                for g in range(NGRP):
                    _step2_group(nc, pools, st, g)
                    if g == 2 and st_next is not None:
                        # inject the next set's first step-1 group so the
                        # PE stream stays dense through the step-2 tail
                        _step1_group(nc, pools, st_next, 0)
                st = st_next

    nc.compile()
    return nc


def _shift_mats():
    s_up = np.zeros((128, 128), np.float32)  # out[m] = in[m-1]
    s_dn = np.zeros((128, 128), np.float32)  # out[m] = in[m+1]
    for m in range(128):
        if m >= 1:
            s_up[m - 1, m] = 1.0
        if m <= 126:
            s_dn[m + 1, m] = 1.0
    s_up_m = np.zeros((MERGED_NP, MERGED_NP), np.float32)
    s_dn_m = np.zeros((MERGED_NP, MERGED_NP), np.float32)
    for base in (0, MERGED_B1_OFF):
        for m in range(20):
            if m >= 1:
                s_up_m[base + m - 1, base + m] = 1.0
            if m <= 18:
                s_dn_m[base + m + 1, base + m] = 1.0
    return s_up, s_dn, s_up_m, s_dn_m


_NC_CACHE = {}


def kernel(world, rand_movement=None, rand_interact=None, rand_element=None,
           **_ignored):
    world = np.ascontiguousarray(world, dtype=np.float32)
    assert world.shape == (B, C, H, W), world.shape
    if "nc" not in _NC_CACHE:
        _NC_CACHE["nc"] = build_kernel()
    nc = _NC_CACHE["nc"]
    s_up, s_dn, s_up_m, s_dn_m = _shift_mats()
    in_maps = []
    for core in range(N_CORES):
        shard = world[core * B_PER_CORE : (core + 1) * B_PER_CORE]
        in_maps.append({"world": np.ascontiguousarray(shard),
                        "s_up": s_up, "s_dn": s_dn,
                        "s_up_m": s_up_m, "s_dn_m": s_dn_m})
    res = run_bass_kernel_spmd(nc, in_maps, list(range(N_CORES)),
                               trace=_NC_CACHE.get("trace", False))
    _NC_CACHE["last_result"] = res
    out = np.concatenate([r["out"] for r in res.results], axis=0)
    return out.astype(np.float32)


if __name__ == "__main__":
    rng = np.random.default_rng(0)
    w = rng.standard_normal((B, C, H, W)).astype(np.float32)
    w[:, 0] = rng.integers(0, 10, (B, 1, H, W)).astype(np.float32)[:, 0]
    out = kernel(w)
    print("ran:", out.shape, out.dtype)
